# revision 1
# baseline (speedup 1.0000x reference)
"""Trainium2 Bass kernel for nn_DiversityMetric (batched NND diversity metric).

Math (per batch b, X = pred_poses[b] in R^{N x D}, N=2048, D=128):
    sq_dist[i,j] = ||xi||^2 + ||xj||^2 - 2 <xi,xj>, diag = inf
    nnd[i]       = sqrt(min_{j != i} sq_dist[i,j])
    out          = [mean(nnd), std(nnd, ddof=1), cv]   over all B*N points

Device strategy (8 cores, 2 batches/core), all-bf16 data path:
    - Host pre-transposes each batch to XT [D=128, N=2048] (bf16).
    - Per row-block m: PSUM tile [128, 2048] accumulates  g_ij - 0.5*sqn_j
      via 4+4 bf16 matmuls (lhsT = XT m-block / -0.5 fill).
      nnd_i = sqrt(relu(sqn_i - 2*max_{j!=i}(g_ij - 0.5*sqn_j))).
    - The diagonal (j==i) is excluded structurally: the reduction skips the
      m-th 128-wide column window (segmented reduce on path A; masked copy
      on path B) -- no diag-mask matmul needed.
    - The max-reduction (bottleneck: every PSUM element streams through a
      1-elem/cycle port; DVE tensor_reduce is always 1x) is split:
        path A: DVE tensor_reduce straight from PSUM        (~1.04 ns/elem)
        path B: ACT copies PSUM->SBUF bf16 (~0.83 ns/elem), DVE folds with
                tensor_tensor max in bf16 2x mode, then a 512-wide reduce
      so DVE and ACT drain PSUM concurrently.
    - sqn_i columns from tiny N=2 matmuls (lhsT = sq m-block, rhs = ones).
    - Host computes final mean/std/cv from the gathered 16x2048 NND matrix.
"""

import numpy as np
from contextlib import ExitStack

import ml_dtypes

import concourse.bass as bass
import concourse.bacc as bacc
import concourse.mybir as mybir
import concourse.tile as tile
from concourse.bass_utils import run_bass_kernel_spmd

F32 = mybir.dt.float32
BF16 = mybir.dt.bfloat16

B, N, D = 16, 2048, 128
NCORES = 8
BPC = B // NCORES          # batches per core
P = 128                    # partitions
MBLK = N // P              # 16 row blocks per batch
MMW = 512                  # matmul moving width (1 PSUM bank)
CHUNK = 1024               # xt/sq SBUF chunk width
NEGBIG = -1.0e6

# epilogue path per row index: 'A' = direct DVE reduce, 'B' = ACT+DVE fold
PATTERN = ['A' if i % 6 == 0 else 'B' for i in range(BPC * MBLK)]

_CACHE = {}


def build_kernel():
    nc = bacc.Bacc("TRN2", target_bir_lowering=False, debug=False)

    xt_d = nc.dram_tensor("xt", [BPC, P, N], BF16, kind="ExternalInput")
    neghalf_d = nc.dram_tensor("neghalf", [P, P], BF16, kind="ExternalInput")
    ones_d = nc.dram_tensor("onescol", [P, 2], BF16, kind="ExternalInput")
    identneg_d = nc.dram_tensor("identneg", [P, P], BF16, kind="ExternalInput")
    ident_d = nc.dram_tensor("ident", [P, P], BF16, kind="ExternalInput")
    nnd_d = nc.dram_tensor("nnd", [P, BPC * MBLK], F32, kind="ExternalOutput")

    with tile.TileContext(nc) as tc, ExitStack() as ctx:
        const = ctx.enter_context(tc.tile_pool(name="const", bufs=1))
        xpool = ctx.enter_context(tc.tile_pool(name="x", bufs=1))
        spool = ctx.enter_context(tc.tile_pool(name="s", bufs=1))
        small = ctx.enter_context(tc.tile_pool(name="small", bufs=1))
        cpool = ctx.enter_context(tc.tile_pool(name="cp", bufs=4))
        fpool = ctx.enter_context(tc.tile_pool(name="fold", bufs=3))
        psum = ctx.enter_context(tc.tile_pool(name="psum", bufs=3, space="PSUM"))
        psq = ctx.enter_context(tc.tile_pool(name="psq", bufs=1, space="PSUM"))

        # first data chunk ASAP on the sync DMA queue; consts go on the
        # scalar-engine HWDGE queue so they don't delay the chunk chain
        NCHUNK = N // CHUNK
        xts = {}
        sqs = {}

        def load_chunk(b, c):
            xtile = xpool.tile([P, CHUNK], BF16, tag=f"xt_{b}_{c}")
            nc.sync.dma_start(
                xtile[:], xt_d.ap()[b, :, c * CHUNK:(c + 1) * CHUNK]
            )
            xts[(b, c)] = xtile
            stile = spool.tile([P, CHUNK], BF16, tag=f"sq_{b}_{c}")
            nc.scalar.square(stile[:], xtile[:])
            sqs[(b, c)] = stile

        load_chunk(0, 0)

        neghalf = const.tile([P, P], BF16)
        nc.scalar.dma_start(neghalf[:], neghalf_d.ap())
        onescol = const.tile([P, 2], BF16)
        nc.scalar.dma_start(onescol[:], ones_d.ap())
        identneg = const.tile([P, P], BF16)
        nc.scalar.dma_start(identneg[:], identneg_d.ap())
        ident = const.tile([P, P], BF16)
        nc.scalar.dma_start(ident[:], ident_d.ap())

        for b in range(BPC):
            for c in range(NCHUNK):
                if (b, c) != (0, 0):
                    load_chunk(b, c)

        def xcol(b, j0, w):
            c = j0 // CHUNK
            off = j0 - c * CHUNK
            assert off + w <= CHUNK
            return xts[(b, c)][:, off:off + w]

        def scol(b, j0, w):
            c = j0 // CHUNK
            off = j0 - c * CHUNK
            assert off + w <= CHUNK
            return sqs[(b, c)][:, off:off + w]

        rmax2 = small.tile([P, BPC * MBLK, 2], F32)
        nc.gpsimd.memset(rmax2[:], -1.0e30)

        # main loop: two [128,1024] psum tiles per (batch, row-block)
        for b in range(BPC):
            for m in range(MBLK):
                col = b * MBLK + m
                path = PATTERN[col]
                lhs_x = xcol(b, m * P, P)
                phs = []
                for h in range(2):
                    ph = psum.tile([P, N // 2], F32, tag="ph")
                    phs.append(ph)
                    diag_k = (m * P) // MMW
                    for k in range(2):
                        j0 = h * (N // 2) + k * MMW
                        nc.tensor.matmul(
                            ph[:, k * MMW:(k + 1) * MMW],
                            lhs_x,
                            xcol(b, j0, MMW),
                            start=True, stop=False,
                        )
                    for k in range(2):
                        j0 = h * (N // 2) + k * MMW
                        diag_here = diag_k == j0 // MMW
                        nc.tensor.matmul(
                            ph[:, k * MMW:(k + 1) * MMW],
                            neghalf[:],
                            scol(b, j0, MMW),
                            start=False, stop=not diag_here,
                        )
                        if diag_here:
                            off = m * P - h * (N // 2)
                            nc.tensor.matmul(
                                ph[:, off:off + P],
                                identneg[:],
                                ident[:],
                                start=False, stop=True,
                            )
                if path == 'A':
                    for h in range(2):
                        nc.vector.tensor_reduce(
                            rmax2[:, col, h:h + 1], phs[h][:],
                            axis=mybir.AxisListType.X, op=mybir.AluOpType.max,
                        )
                else:
                    cps = []
                    for h in range(2):
                        cp = cpool.tile([P, N // 2], BF16, tag="cp")
                        nc.scalar.copy(cp[:], phs[h][:])
                        cps.append(cp)
                    t1 = fpool.tile([P, N // 2], BF16, tag="bt1")
                    nc.vector.tensor_tensor(
                        t1[:], cps[0][:], cps[1][:], op=mybir.AluOpType.max
                    )
                    t2 = fpool.tile([P, N // 4], BF16, tag="bt2")
                    nc.vector.tensor_tensor(
                        t2[:], t1[:, :N // 4], t1[:, N // 4:],
                        op=mybir.AluOpType.max,
                    )
                    nc.vector.tensor_reduce(
                        rmax2[:, col, 0:1], t2[:],
                        axis=mybir.AxisListType.X, op=mybir.AluOpType.max,
                    )

        # sqn_i columns: tiny matmuls (emitted last; reuses a psum slot)
        psum_sqn = psq.tile([P, 2 * BPC * MBLK], F32)
        for b in range(BPC):
            for m in range(MBLK):
                col = b * MBLK + m
                nc.tensor.matmul(
                    psum_sqn[:, 2 * col:2 * col + 2],
                    scol(b, m * P, P),
                    onescol[:],
                    start=True, stop=True,
                )
        sqn_cols = small.tile([P, BPC * MBLK], F32)
        nc.vector.tensor_copy(
            sqn_cols[:],
            psum_sqn[:].rearrange("p (c t) -> p c t", t=2)[:, :, 0:1],
        )

        # nnd = sqrt(relu(sqn_i + (-2)*max))
        rmax = small.tile([P, BPC * MBLK], F32)
        nc.vector.tensor_reduce(
            rmax[:], rmax2[:], axis=mybir.AxisListType.X, op=mybir.AluOpType.max
        )
        nnd2 = small.tile([P, BPC * MBLK], F32)
        nc.vector.scalar_tensor_tensor(
            nnd2[:], rmax[:], -2.0, sqn_cols[:],
            op0=mybir.AluOpType.mult, op1=mybir.AluOpType.add,
        )
        nc.vector.tensor_scalar_max(nnd2[:], nnd2[:], 0.0)
        nnd_sb = small.tile([P, BPC * MBLK], F32)
        nc.scalar.sqrt(nnd_sb[:], nnd2[:])
        nc.sync.dma_start(nnd_d.ap()[:, :], nnd_sb[:])

    nc.compile()
    return nc


def _consts():
    neghalf = np.full((P, P), -0.5, dtype=ml_dtypes.bfloat16)
    onescol = np.ones((P, 2), dtype=ml_dtypes.bfloat16)
    identneg = (NEGBIG * np.eye(P)).astype(ml_dtypes.bfloat16)
    ident = np.eye(P, dtype=np.float32).astype(ml_dtypes.bfloat16)
    return neghalf, onescol, identneg, ident


def make_in_maps(pred_poses):
    neghalf, onescol, identneg, ident = _consts()
    in_maps = []
    for c in range(NCORES):
        xb = pred_poses[c * BPC:(c + 1) * BPC]
        xt = np.ascontiguousarray(
            xb.transpose(0, 2, 1)).astype(ml_dtypes.bfloat16)
        in_maps.append({
            "xt": xt, "neghalf": neghalf, "onescol": onescol,
            "identneg": identneg, "ident": ident,
        })
    return in_maps


def kernel(pred_poses: np.ndarray) -> np.ndarray:
    pred_poses = np.ascontiguousarray(np.asarray(pred_poses, dtype=np.float32))
    assert pred_poses.shape == (B, N, D)

    if "nc" not in _CACHE:
        _CACHE["nc"] = build_kernel()
    nc = _CACHE["nc"]

    in_maps = make_in_maps(pred_poses)
    res = run_bass_kernel_spmd(nc, in_maps, list(range(NCORES)))

    nnd = np.zeros((B, N), dtype=np.float64)
    for c in range(NCORES):
        t = np.asarray(res.results[c]["nnd"])           # [128, BPC*MBLK]
        for bl in range(BPC):
            sub = t[:, bl * MBLK:(bl + 1) * MBLK]       # [128, 16] (p, m)
            nnd[c * BPC + bl] = sub.T.reshape(N)        # index m*128+p

    mean = nnd.mean()
    std = nnd.std(ddof=1)
    eps = 1e-8
    cv = std / max(mean, eps) if mean > eps else 0.0
    return np.stack([mean, std, cv]).astype(np.float32)

